# revision 1
# baseline (speedup 1.0000x reference)
"""Trainium2 Bass kernel for nn_EnergyMovers (batched Sinkhorn OT loss).

Strategy (pure data parallelism, 4 batch elems per core x 8 cores):
  - Host: build masked augmented point vectors so d2[n,m] = sum_k A[k,n]*B[k,m]
    comes out of a K=4 TensorE matmul already masked (masked rows/cols -> d2=0
    -> K=exp(-sqrt(1e-12)/eps) ~ 1, matching the reference's logK=0 there).
  - Device per elem: d2 (both layouts) -> clamp(DVE) -> sqrt(ACT) -> exp(ACT)
    giving K (layout A [n,m]), KT (layout B [m,n]) and D2KT = s^2*K (layout B)
    resident in SBUF. Then 50 non-log Sinkhorn iterations as PE matvecs with
    the potential vector as the 1-column stationary operand:
        u = aw * recip(K @ v),  v = bw * recip(K.T @ u)
    (mathematically identical to the reference's log-domain iteration; f32
    potentials stay in range: max |v| ~ 1e18 over 50 iters).
  - Final: ot = u . (D2KT.T @ v) via one more matvec + ones-matmul reduction.
  - Host: huber(e) added, results gathered from 8 cores.
"""

import os
from contextlib import ExitStack

import numpy as np

import concourse.bass as bass
import concourse.bacc as bacc
import concourse.mybir as mybir
import concourse.tile as tile
from concourse.bass_utils import run_bass_kernel_spmd

N_CORES = 8
ELEMS = 4  # batch elements per core (B=32 / 8)
B, N, M = 32, 512, 512
EPS = 0.05
ITERS = int(os.environ.get("EM_ITERS", "50"))
F32 = mybir.dt.float32
AF = mybir.ActivationFunctionType


def _build_nc():
    nc = bacc.Bacc()
    # single param per purpose so each elem's SBUF load is ONE dma (one HWDGE
    # sem) — walrus allows at most 2 sync waits per Matmult instruction.
    ABaug = nc.declare_dram_parameter("ABaug", [ELEMS, 4, 2 * N],
                                      mybir.dt.float32r, isOutput=False)
    wtsp = nc.declare_dram_parameter("wts", [ELEMS, 128, 8], F32, isOutput=False)
    otp = nc.declare_dram_parameter("ot", [1, ELEMS], F32, isOutput=True)

    with ExitStack() as ctx:
        tc = ctx.enter_context(tile.TileContext(nc))
        kpool = ctx.enter_context(tc.tile_pool(name="kmat", bufs=1))
        tpool = ctx.enter_context(tc.tile_pool(name="tmp", bufs=2))
        vpool = ctx.enter_context(tc.tile_pool(name="vec", bufs=1))
        spool = ctx.enter_context(tc.tile_pool(name="sf", bufs=4))
        pd2 = ctx.enter_context(tc.tile_pool(name="pd2", bufs=2, space="PSUM"))
        pss = ctx.enter_context(tc.tile_pool(name="pss", bufs=3, space="PSUM"))
        pst = ctx.enter_context(tc.tile_pool(name="pst", bufs=3, space="PSUM"))

        ones = vpool.tile([128, 1], F32, tag="ones", name="ones")
        nc.gpsimd.memset(ones[:], 1.0)
        ident = vpool.tile([1, 1], F32, tag="ident", name="ident")
        nc.gpsimd.memset(ident[:], 1.0)
        identb = vpool.tile([1, 1], mybir.dt.bfloat16, tag="identb", name="identb")
        nc.gpsimd.memset(identb[:], 1.0)
        bias12 = vpool.tile([128, 1], F32, tag="bias12", name="bias12")
        nc.gpsimd.memset(bias12[:], 1e-12)
        outsb = vpool.tile([1, ELEMS], F32, tag="outsb", name="outsb")

        KA, KB, DK, U, V, AW, BW, AB_SB = {}, {}, {}, {}, {}, {}, {}, {}
        for e in range(ELEMS):
            # f32r so the K=4 d2 matmuls stream at 1 cycle/row (fp32 is 4)
            ab_sb = vpool.tile([4, 2 * N], mybir.dt.float32r,
                               tag=f"ABs{e}", name=f"ABs{e}")
            nc.sync.dma_start(out=ab_sb[:], in_=ABaug[e])
            a_sb = ab_sb[:, 0:N]
            b_sb = ab_sb[:, N:2 * N]
            wt_sb = vpool.tile([128, 8], F32, tag=f"wt{e}", name=f"wt{e}")
            nc.sync.dma_start(out=wt_sb[:], in_=wtsp[e])
            AW[e] = wt_sb[:, 0:4]
            BW[e] = wt_sb[:, 4:8]
            U[e] = vpool.tile([128, 4], mybir.dt.bfloat16, tag=f"u{e}", name=f"u{e}")
            V[e] = vpool.tile([128, 4], mybir.dt.bfloat16, tag=f"v{e}", name=f"v{e}")
            nc.gpsimd.memset(V[e][:], 1.0)
            KA[e], KB[e], DK[e] = [], [], []
            AB_SB[e] = (a_sb, b_sb)

        # Setup pass 1: d2 matmuls + clamp + sqrt for ALL tiles, then pass 2:
        # all exps — sqrt and exp live in different ACT table sets, so
        # alternating them reloads the tables (~1.3us) per tile.
        ST = {}
        last_sqrt = None
        for e in range(ELEMS):
            a_sb, b_sb = AB_SB[e]
            for side in ("B", "A"):
                Lt, Rt = (b_sb, a_sb) if side == "B" else (a_sb, b_sb)
                for c in range(4):
                    d2 = pd2.tile([128, 512], F32, tag="d2", name="d2")
                    nc.tensor.matmul(
                        d2[:], Lt[:, c * 128:(c + 1) * 128], Rt[:],
                        start=True, stop=True,
                    )
                    cl = tpool.tile([128, 512], F32, tag="cl", name="cl")
                    nc.vector.tensor_scalar_max(cl[:], d2[:], 0.0)
                    st = tpool.tile([128, 512], F32, tag=f"st{e}{side}{c}",
                                    name=f"st{e}{side}{c}", bufs=1)
                    last_sqrt = nc.scalar.activation(
                        st[:], cl[:], AF.Sqrt, bias=bias12[:]
                    )
                    ST[(e, side, c)] = st
        for e in range(ELEMS):
            for side in ("B", "A"):
                for c in range(4):
                    st = ST[(e, side, c)]
                    # K tiles stored as bf16: full-rate PE streaming and
                    # background-buffer LDWEIGHTS
                    kt = kpool.tile([128, 512], mybir.dt.bfloat16,
                                    tag=f"K{side}{e}c{c}", name=f"K{side}{e}c{c}")
                    exp_inst = nc.scalar.activation(
                        kt[:], st[:], AF.Exp, scale=-1.0 / EPS
                    )
                    # keep all Sqrts before all Exps on ACT: they live in
                    # different table sets; interleaving reloads ~1.3us/op
                    tile.add_dep_helper(
                        exp_inst.ins, last_sqrt.ins,
                        sync=True, reason="act-table-batch",
                    )
                    (KB[e] if side == "B" else KA[e]).append(kt)
                    if side == "B":
                        t1 = tpool.tile([128, 512], F32, tag="t1", name="t1")
                        nc.vector.tensor_mul(t1[:], st[:], kt[:])
                        dk = kpool.tile([128, 512], F32, tag=f"DK{e}c{c}",
                                        name=f"DK{e}c{c}")
                        nc.vector.tensor_mul(dk[:], st[:], t1[:])
                        DK[e].append(dk)

        def matvec_head(rhs_tiles, wvec, bf=True, on_dve=False):
            """pt[128,4] (partition-major PSUM) = sum_c wvec[:,c]^T @ rhs[c].

            bf=True: den vector round-trips through bf16 on the ACT copy and
            PE transposes (iteration path only; error damped by Sinkhorn's
            marginal constraints). Final reduction uses bf=False.
            on_dve: route the PSUM evacuation copy to DVE instead of ACT to
            split the copy load between the two idle-ish engines.
            """
            dt = mybir.dt.bfloat16 if bf else F32
            idn = identb if bf else ident
            ps = pss.tile([1, 512], F32, tag="ps", name="ps")
            for c in range(4):
                nc.tensor.matmul(
                    ps[:], wvec[:, c:c + 1], rhs_tiles[c][:],
                    start=(c == 0), stop=(c == 3),
                )
            sf = spool.tile([1, 512], dt, tag="sf", name="sf")
            if on_dve:
                nc.vector.tensor_copy(sf[:], ps[:])
            else:
                nc.scalar.copy(sf[:], ps[:])
            if bf:
                # bf16 PSUM writes must be 4B-aligned: use every other column
                pt = pst.tile([128, 8], dt, tag="pt", name="pt")
                for c in range(4):
                    nc.tensor.transpose(
                        pt[:, 2 * c:2 * c + 1], sf[0:1, c * 128:(c + 1) * 128],
                        idn[:],
                    )
                return pt.rearrange("p (c t) -> p c t", t=2)[:, :, 0]
            pt = pst.tile([128, 4], dt, tag="pt", name="pt")
            for c in range(4):
                nc.tensor.transpose(
                    pt[:, c:c + 1], sf[0:1, c * 128:(c + 1) * 128], idn[:]
                )
            return pt

        def phase_tail(pm, weight, out_vec):
            rc = spool.tile([128, 4], F32, tag="rc", name="rc")
            nc.vector.reciprocal(rc[:], pm[:])
            nc.vector.tensor_mul(out_vec[:], rc[:], weight[:])

        for _ in range(ITERS):
            sfu = [matvec_head(KB[e], V[e]) for e in range(ELEMS)]
            for e in range(ELEMS):
                phase_tail(sfu[e], AW[e], U[e])
            sfv = [matvec_head(KA[e], U[e]) for e in range(ELEMS)]
            for e in range(ELEMS):
                phase_tail(sfv[e], BW[e], V[e])

        # final: ot[e] = u . (D2KT.T @ v) — full fp32 (errors here hit the
        # output directly, no fixed-point self-correction)
        UVf = {}
        for e in range(ELEMS):
            uf = vpool.tile([128, 4], F32, tag=f"uf{e}", name=f"uf{e}")
            vf = vpool.tile([128, 4], F32, tag=f"vf{e}", name=f"vf{e}")
            nc.vector.tensor_copy(uf[:], U[e][:])
            nc.vector.tensor_copy(vf[:], V[e][:])
            UVf[e] = (uf, vf)
        sfg = [matvec_head(DK[e], UVf[e][1], bf=False) for e in range(ELEMS)]
        for e in range(ELEMS):
            w = spool.tile([128, 4], F32, tag="rc", name="rc")
            nc.vector.tensor_mul(w[:], sfg[e][:], UVf[e][0][:])
            ws = vpool.tile([128, 1], F32, tag=f"ws{e}", name=f"ws{e}")
            nc.vector.reduce_sum(ws[:], w[:], axis=mybir.AxisListType.X)
            po = pst.tile([1, 1], F32, tag="pt", name="po")
            nc.tensor.matmul(po[:], ones[:], ws[:], start=True, stop=True)
            nc.scalar.copy(outsb[0:1, e:e + 1], po[:])
        nc.sync.dma_start(out=otp[:], in_=outsb[:])
    nc.compile()
    return nc


_NC_CACHE = {}


def _get_nc():
    if "nc" not in _NC_CACHE:
        _NC_CACHE["nc"] = _build_nc()
    return _NC_CACHE["nc"]


def _host_prep(a_mask, pc_a, b_mask, pc_b):
    """Per-batch-element f32 prep mirroring the reference's masking."""
    f32 = np.float32
    a_pt = (a_mask * pc_a[..., 2]).astype(f32)          # [B,N]
    b_pt = (b_mask * pc_b[..., 2]).astype(f32)          # [B,M]
    va = (a_pt > 0).astype(f32)
    vb = (b_pt > 0).astype(f32)
    aw = (a_pt / a_pt.sum(axis=1, keepdims=True, dtype=f32)).astype(f32)
    bw = (b_pt / b_pt.sum(axis=1, keepdims=True, dtype=f32)).astype(f32)
    xa = pc_a[..., :2].astype(f32)                      # [B,N,2]
    xb = pc_b[..., :2].astype(f32)
    onesN = np.ones((B, N), f32)
    A = np.stack(
        [-2 * xa[..., 0], -2 * xa[..., 1],
         (xa * xa).sum(-1).astype(f32), onesN], axis=1
    ) * va[:, None, :]                                  # [B,4,N]
    Bm = np.stack(
        [xb[..., 0], xb[..., 1], onesN,
         (xb * xb).sum(-1).astype(f32)], axis=1
    ) * vb[:, None, :]                                  # [B,4,M]
    # huber term on host (tiny)
    e = (a_pt.sum(axis=1, dtype=f32) - b_pt.sum(axis=1, dtype=f32)).astype(f32)
    hub = np.where(np.abs(e) <= 1.0, f32(0.5) * e * e, np.abs(e) - f32(0.5))
    # partition-major chunk layout for [512] vectors: pm[p, c] = vec[128c + p]
    aw_pm = aw.reshape(B, 4, 128).transpose(0, 2, 1).astype(f32)
    bw_pm = bw.reshape(B, 4, 128).transpose(0, 2, 1).astype(f32)
    AB = np.concatenate([A.astype(f32), Bm.astype(f32)], axis=2)  # [B,4,1024]
    wts = np.concatenate([aw_pm, bw_pm], axis=2)                  # [B,128,8]
    return AB, wts, hub.astype(f32)


def kernel(a_mask, pc_a, b_mask, pc_b, _trace=False):
    AB, wts, hub = _host_prep(
        np.asarray(a_mask), np.asarray(pc_a), np.asarray(b_mask), np.asarray(pc_b)
    )
    in_maps = []
    for core in range(N_CORES):
        sl = slice(core * ELEMS, (core + 1) * ELEMS)
        in_maps.append({
            "ABaug": np.ascontiguousarray(AB[sl]),
            "wts": np.ascontiguousarray(wts[sl]),
        })
    nc = _get_nc()
    res = run_bass_kernel_spmd(nc, in_maps, list(range(N_CORES)), trace=_trace)
    ot = np.concatenate([res.results[c]["ot"].reshape(ELEMS) for c in range(N_CORES)])
    out = (ot + hub).astype(np.float32)
    if _trace:
        return out, res
    return out



# revision 8
# speedup vs baseline: 1.3262x; 1.3262x over previous
"""Trainium2 Bass kernel for nn_EnergyMovers (batched Sinkhorn OT loss).

Strategy (pure data parallelism, 4 batch elems per core x 8 cores):
  - Host: build masked augmented point vectors so d2[n,m] = sum_k A[k,n]*B[k,m]
    comes out of a K=4 TensorE matmul already masked (masked rows/cols -> d2=0
    -> K=exp(-sqrt(1e-12)/eps) ~ 1, matching the reference's logK=0 there).
  - Device per elem: d2 (layout A only) -> clamp(DVE) -> sqrt(ACT) -> exp(ACT)
    giving KA (layout [n,m]); KB ([m,n]) comes from 128x128 PE transposes of
    KA; DK = clamp(d2)*KA (= d^2*K up to the 1e-12 sqrt bias) stays layout A.
  - 50 non-log Sinkhorn iterations, mathematically identical to the
    reference's log-domain iteration (f32/bf16 exponent range suffices:
    max |v| ~ 1e18 over 50 iters):
        u = aw * recip(K @ v),  v = bw * recip(K.T @ u)
    Each matvec is COLUMN-TILED on the PE: the 4 batch elems' matmuls go to
    the 4 distinct 32-wide PE column groups (tile_position derived from the
    PSUM output base partition 32e), so their K-matrix streams run
    concurrently (~4x over serial M=1 matvecs). The M=32 stationary has the
    potential chunk in col 0 and zeros in cols 1-31 (also zero-fills the
    unused PSUM rows so downstream full-tile reads see no garbage).
    Row->partition-major conversion uses 4 fat [128,128] PE transposes per
    phase (all 4 elems at once), then a single strided DVE divide
    U[:, c, e] = aw[:, c, e] / s[:, c, e] produces the next stationaries.
  - Final: ot[e] = (DK^T u) . v via one more column-tiled matvec + transposes
    + DVE mul/reduce + ones-matmul partition reduction.
  - Host: huber(e) added, results gathered from 8 cores.
"""

import os
from contextlib import ExitStack

import numpy as np

import concourse.bass as bass
import concourse.bacc as bacc
import concourse.mybir as mybir
import concourse.tile as tile
from concourse.bass_utils import run_bass_kernel_spmd

N_CORES = 8
ELEMS = 4  # batch elements per core (B=32 / 8)
B, N, M = 32, 512, 512
EPS = 0.05
ITERS = int(os.environ.get("EM_ITERS", "50"))
F32 = mybir.dt.float32
BF16 = mybir.dt.bfloat16
AF = mybir.ActivationFunctionType
ALU = mybir.AluOpType


def _build_nc():
    nc = bacc.Bacc()
    ABaug = nc.declare_dram_parameter("ABaug", [ELEMS, 4, 2 * N],
                                      mybir.dt.float32r, isOutput=False)
    # wts[p, 0:16] = aw[(c,e)] chunks, wts[p, 16:32] = bw[(c,e)] chunks
    wtsp = nc.declare_dram_parameter("wts", [128, 32], F32, isOutput=False)
    eyep = nc.declare_dram_parameter("eye", [128, 128], F32, isOutput=False)
    otp = nc.declare_dram_parameter("ot", [1, ELEMS], F32, isOutput=True)

    with ExitStack() as ctx:
        tc = ctx.enter_context(tile.TileContext(nc))
        # persistent SBUF
        kpool = ctx.enter_context(tc.tile_pool(name="kmat", bufs=1))
        vpool = ctx.enter_context(tc.tile_pool(name="vec", bufs=1))

        # --- load params -------------------------------------------------
        wt_sb = vpool.tile([128, 32], F32, tag="wt", name="wt")
        nc.sync.dma_start(out=wt_sb[:], in_=wtsp[:])
        aw_v = wt_sb[:, 0:16].rearrange("p (c e) -> p c e", c=4)   # [128,4,4]
        bw_v = wt_sb[:, 16:32].rearrange("p (c e) -> p c e", c=4)
        eye_sb = vpool.tile([128, 128], F32, tag="eyef", name="eyef")
        nc.sync.dma_start(out=eye_sb[:], in_=eyep[:])
        identB = vpool.tile([128, 128], BF16, tag="identB", name="identB")
        nc.vector.tensor_copy(identB[:], eye_sb[:])
        ones = vpool.tile([128, 1], F32, tag="ones", name="ones")
        nc.gpsimd.memset(ones[:], 1.0)
        ones16 = vpool.tile([128, 16], F32, tag="ones16", name="ones16")
        nc.gpsimd.memset(ones16[:], 1.0)
        bias12 = vpool.tile([128, 1], F32, tag="bias12", name="bias12")
        nc.gpsimd.memset(bias12[:], 1e-12)
        outsb = vpool.tile([1, ELEMS], F32, tag="outsb", name="outsb")

        # potentials: [128, (c, e, 32)] bf16; col 0 of each 32-block is the
        # live value, cols 1-31 stay zero (zero-pads the M=32 stationary).
        U_all = vpool.tile([128, 4, 4, 32], BF16, tag="U", name="U")
        V_all = vpool.tile([128, 4, 4, 32], BF16, tag="V", name="V")
        nc.gpsimd.memset(U_all[:], 0.0)
        nc.gpsimd.memset(V_all[:], 0.0)
        nc.vector.tensor_copy(
            V_all[:, :, :, 0], ones16[:].rearrange("p (c e) -> p c e", c=4)
        )

        KA, KB, DK, AB_SB = {}, {}, {}, {}
        for e in range(ELEMS):
            ab_sb = vpool.tile([4, 2 * N], mybir.dt.float32r,
                               tag=f"ABs{e}", name=f"ABs{e}")
            nc.sync.dma_start(out=ab_sb[:], in_=ABaug[e])
            AB_SB[e] = (ab_sb[:, 0:N], ab_sb[:, N:2 * N])
            KA[e], KB[e], DK[e] = [], [], []
            for c in range(4):
                KA[e].append(kpool.tile([128, 512], BF16, tag=f"KA{e}{c}",
                                        name=f"KA{e}{c}"))
                KB[e].append(kpool.tile([128, 512], BF16, tag=f"KB{e}{c}",
                                        name=f"KB{e}{c}"))
                DK[e].append(kpool.tile([128, 512], BF16, tag=f"DK{e}{c}",
                                        name=f"DK{e}{c}"))

        # --- setup: build KA (exp(-d/eps)), DK (d^2*K), KB (transpose) ---
        with tc.tile_pool(name="pd2", bufs=2, space="PSUM") as pd2, \
             tc.tile_pool(name="ptp", bufs=2, space="PSUM") as ptp, \
             tc.tile_pool(name="cl", bufs=1) as clpool, \
             tc.tile_pool(name="st", bufs=1) as stpool:
            CL, ST = {}, {}
            last_sqrt = None
            for e in range(ELEMS):
                a_sb, b_sb = AB_SB[e]
                for c in range(4):
                    d2 = pd2.tile([128, 512], F32, tag="d2", name="d2")
                    nc.tensor.matmul(
                        d2[:], a_sb[:, c * 128:(c + 1) * 128], b_sb[:],
                        start=True, stop=True,
                    )
                    cl = clpool.tile([128, 512], F32, tag=f"cl{e}{c}",
                                     name=f"cl{e}{c}")
                    nc.vector.tensor_scalar_max(cl[:], d2[:], 0.0)
                    st = stpool.tile([128, 512], F32, tag=f"st{e}{c}",
                                     name=f"st{e}{c}")
                    last_sqrt = nc.scalar.activation(
                        st[:], cl[:], AF.Sqrt, bias=bias12[:]
                    )
                    CL[(e, c)] = cl
                    ST[(e, c)] = st
            # all exps after all sqrts (ACT table sets differ; interleaving
            # reloads ~1.3us per switch)
            for e in range(ELEMS):
                for c in range(4):
                    exp_inst = nc.scalar.activation(
                        KA[e][c][:], ST[(e, c)][:], AF.Exp, scale=-1.0 / EPS
                    )
                    tile.add_dep_helper(exp_inst.ins, last_sqrt.ins,
                                        sync=True, reason="act-table-batch")
            for e in range(ELEMS):
                for c in range(4):
                    # DK = d^2 * K (st^2 == cl + 1e-12; the 1e-12 is noise)
                    nc.vector.tensor_mul(DK[e][c][:], CL[(e, c)][:],
                                         KA[e][c][:])
                    # KB[cj][:, 128*ci:...] = KA[ci][:, 128*cj:...]^T
                    for cj in range(4):
                        tp = ptp.tile([128, 128], BF16, tag="tp", name="tp")
                        nc.tensor.transpose(
                            tp[:], KA[e][c][:, cj * 128:(cj + 1) * 128],
                            identB[:],
                        )
                        nc.vector.tensor_copy(
                            KB[e][cj][:, c * 128:(c + 1) * 128], tp[:]
                        )

        # --- Sinkhorn iterations ----------------------------------------
        with tc.tile_pool(name="ps", bufs=2, space="PSUM") as pspool, \
             tc.tile_pool(name="pt", bufs=2, space="PSUM") as ptpool, \
             tc.tile_pool(name="sb", bufs=2) as sbpool:

            def phase(Kt, stat, wv, out_all):
                """out_all[:,c,e,0] = wv[:,c,e] / (sum_c stat[:,c,e,:]^T Kt)"""
                ps = pspool.tile([128, 512], F32, tag="ps", name="ps")
                for c in range(4):
                    for e in range(ELEMS):
                        nc.tensor.matmul(
                            ps[32 * e:32 * e + 32, :],
                            stat[:, c, e, :], Kt[e][c][:],
                            start=(c == 0), stop=(c == 3),
                            tile_position=(0, 32 * e),
                        )
                sbu = sbpool.tile([128, 512], BF16, tag="sbu", name="sbu")
                pt = ptpool.tile([128, 4, 128, 2], BF16, tag="pt", name="pt")
                for c in range(4):
                    nc.vector.tensor_copy(
                        sbu[:, c * 128:(c + 1) * 128],
                        ps[:, c * 128:(c + 1) * 128],
                    )
                    nc.tensor.transpose(
                        pt[:, c, :, 0], sbu[:, c * 128:(c + 1) * 128],
                        identB[:],
                    )
                # s values for (c, elem e) live at pt[:, c, 32e, 0]
                s_v = pt.rearrange("p c (g r) t -> p c g r t", g=4)[:, :, :, 0, 0]
                rc = sbpool.tile([128, 4, 4], F32, tag="rc", name="rc")
                nc.vector.reciprocal(rc[:], s_v)
                nc.vector.tensor_mul(out_all[:, :, :, 0], rc[:], wv)

            for _ in range(ITERS):
                phase(KB, V_all, aw_v, U_all)   # u = aw / (K @ v)
                phase(KA, U_all, bw_v, V_all)   # v = bw / (K^T @ u)

            # --- final: ot[e] = (DK^T u) . v ----------------------------
            ps = pspool.tile([128, 512], F32, tag="ps", name="ps")
            for c in range(4):
                for e in range(ELEMS):
                    nc.tensor.matmul(
                        ps[32 * e:32 * e + 32, :],
                        U_all[:, c, e, :], DK[e][c][:],
                        start=(c == 0), stop=(c == 3),
                        tile_position=(0, 32 * e),
                    )
            sbg = sbpool.tile([128, 512], BF16, tag="sbu", name="sbg")
            ptg = ptpool.tile([128, 4, 128, 2], BF16, tag="pt", name="ptg")
            for c in range(4):
                nc.vector.tensor_copy(
                    sbg[:, c * 128:(c + 1) * 128],
                    ps[:, c * 128:(c + 1) * 128],
                )
                nc.tensor.transpose(
                    ptg[:, c, :, 0], sbg[:, c * 128:(c + 1) * 128], identB[:]
                )
            g_v = ptg.rearrange("p c (g r) t -> p c g r t", g=4)[:, :, :, 0, 0]
            t_ce = sbpool.tile([128, 4, 4], F32, tag="tce", name="tce")
            nc.vector.tensor_tensor(
                out=t_ce[:], in0=g_v, in1=V_all[:, :, :, 0], op=ALU.mult
            )
            r_e = sbpool.tile([128, 4], F32, tag="re", name="re")
            nc.vector.reduce_sum(
                r_e[:], t_ce[:].rearrange("p c e -> p e c"),
                axis=mybir.AxisListType.X,
            )
            po = ptpool.tile([1, 4], F32, tag="pt", name="po")
            nc.tensor.matmul(po[:], ones[:], r_e[:], start=True, stop=True)
            nc.scalar.copy(outsb[:], po[:])
            nc.sync.dma_start(out=otp[:], in_=outsb[:])
    nc.compile()
    return nc


_NC_CACHE = {}


def _get_nc():
    if "nc" not in _NC_CACHE:
        _NC_CACHE["nc"] = _build_nc()
    return _NC_CACHE["nc"]


def _host_prep(a_mask, pc_a, b_mask, pc_b):
    """Per-batch-element f32 prep mirroring the reference's masking."""
    f32 = np.float32
    a_pt = (a_mask * pc_a[..., 2]).astype(f32)          # [B,N]
    b_pt = (b_mask * pc_b[..., 2]).astype(f32)          # [B,M]
    va = (a_pt > 0).astype(f32)
    vb = (b_pt > 0).astype(f32)
    aw = (a_pt / a_pt.sum(axis=1, keepdims=True, dtype=f32)).astype(f32)
    bw = (b_pt / b_pt.sum(axis=1, keepdims=True, dtype=f32)).astype(f32)
    xa = pc_a[..., :2].astype(f32)                      # [B,N,2]
    xb = pc_b[..., :2].astype(f32)
    onesN = np.ones((B, N), f32)
    A = np.stack(
        [-2 * xa[..., 0], -2 * xa[..., 1],
         (xa * xa).sum(-1).astype(f32), onesN], axis=1
    ) * va[:, None, :]                                  # [B,4,N]
    Bm = np.stack(
        [xb[..., 0], xb[..., 1], onesN,
         (xb * xb).sum(-1).astype(f32)], axis=1
    ) * vb[:, None, :]                                  # [B,4,M]
    # huber term on host (tiny)
    e = (a_pt.sum(axis=1, dtype=f32) - b_pt.sum(axis=1, dtype=f32)).astype(f32)
    hub = np.where(np.abs(e) <= 1.0, f32(0.5) * e * e, np.abs(e) - f32(0.5))
    # [B, 4, 128]: chunk-major potentials/weights
    aw_pm = aw.reshape(B, 4, 128).astype(f32)
    bw_pm = bw.reshape(B, 4, 128).astype(f32)
    AB = np.concatenate([A.astype(f32), Bm.astype(f32)], axis=2)  # [B,4,1024]
    return AB, aw_pm, bw_pm, hub.astype(f32)


def kernel(a_mask, pc_a, b_mask, pc_b, _trace=False):
    AB, aw_pm, bw_pm, hub = _host_prep(
        np.asarray(a_mask), np.asarray(pc_a), np.asarray(b_mask), np.asarray(pc_b)
    )
    eye = np.eye(128, dtype=np.float32)
    in_maps = []
    for core in range(N_CORES):
        sl = slice(core * ELEMS, (core + 1) * ELEMS)
        # wts[p, (c, e)] = w_{core*4+e}[128c + p]
        awc = aw_pm[sl].transpose(2, 1, 0).reshape(128, 16)  # [p, c, e]
        bwc = bw_pm[sl].transpose(2, 1, 0).reshape(128, 16)
        in_maps.append({
            "ABaug": np.ascontiguousarray(AB[sl]),
            "wts": np.ascontiguousarray(
                np.concatenate([awc, bwc], axis=1)),
            "eye": eye,
        })
    nc = _get_nc()
    res = run_bass_kernel_spmd(nc, in_maps, list(range(N_CORES)), trace=_trace)
    ot = np.concatenate([res.results[c]["ot"].reshape(ELEMS) for c in range(N_CORES)])
    out = (ot + hub).astype(np.float32)
    if _trace:
        return out, res
    return out


# revision 12
# speedup vs baseline: 1.4731x; 1.1108x over previous
"""Trainium2 Bass kernel for nn_EnergyMovers (batched Sinkhorn OT loss).

Strategy (pure data parallelism, 4 batch elems per core x 8 cores):
  - Host: build masked augmented point vectors so d2[n,m] = sum_k A[k,n]*B[k,m]
    comes out of a K=4 TensorE matmul already masked (masked rows/cols -> d2=0
    -> K=exp(-sqrt(1e-12)/eps) ~ 1, matching the reference's logK=0 there).
  - Device per elem: d2 (both layouts) -> clamp(DVE) -> sqrt(ACT) -> exp(ACT)
    with the marginal weights FOLDED INTO THE KERNEL MATRICES via the exp
    bias: K'A = exp(-d/eps + ln(aw_n)) = aw_n * K (layout [n,m]),
    K'B = bw_m * K (layout [m,n]), DK' = clamp(d2) * K'A = aw_n * d^2 * K.
  - Non-log Sinkhorn on reciprocal potentials (u = aw*U~, v = bw*V~):
        s_v = K'A^T @ U~ ; V~ = 1/s_v ; s_u = K'B^T @ V~ ; U~ = 1/s_u
    identical to the reference's log-domain iteration (f32/bf16 exponent
    range suffices). The first u-update is the row-sum of K'A (v0 = 1
    including the reference's masked-column exp(0)=1 terms), done at setup
    with DVE reductions.
  - Each matvec is COLUMN-TILED on the PE: the 4 batch elems go to the 4
    32-wide PE column groups (tile_position (0,32e)), so their K-matrix
    streams run concurrently. The M=32 stationary has the potential chunk in
    col 0, zeros elsewhere (also zero-fills unused PSUM rows).
  - Row -> partition-major conversion via SELECTOR MATMULS: pt[128,4] =
    sbu_chunk^T @ sel where sel[32e,e]=1 picks the 4 result rows. Then ONE
    strided DVE reciprocal [128,(c,e)] produces the next stationaries.
  - Dummy N=64 matmuls pad the PE pipeline during DVE/ACT tail waits so the
    HAM clock gate sees sustained activity and keeps the PE at 2.4 GHz.
  - Final: ot[e] = (DK'^T U~) . (bw * V~) via one more column-tiled matvec +
    selector matmuls + DVE muls + ones-matmul partition reduction.
  - Host: huber(e) added, results gathered from 8 cores.
"""

import os
from contextlib import ExitStack

import numpy as np

import concourse.bass as bass
import concourse.bacc as bacc
import concourse.mybir as mybir
import concourse.tile as tile
from concourse.bass_utils import run_bass_kernel_spmd

N_CORES = 8
ELEMS = 4  # batch elements per core (B=32 / 8)
B, N, M = 32, 512, 512
EPS = 0.05
ITERS = int(os.environ.get("EM_ITERS", "50"))
F32 = mybir.dt.float32
BF16 = mybir.dt.bfloat16
AF = mybir.ActivationFunctionType
ALU = mybir.AluOpType


def _build_nc():
    nc = bacc.Bacc()
    ABaug = nc.declare_dram_parameter("ABaug", [ELEMS, 4, 2 * N],
                                      mybir.dt.float32r, isOutput=False)
    # wts cols: 0:16 aw[(c,e)], 16:32 bw[(c,e)], 32:48 ln_aw, 48:64 ln_bw
    wtsp = nc.declare_dram_parameter("wts", [128, 64], F32, isOutput=False)
    otp = nc.declare_dram_parameter("ot", [1, ELEMS], F32, isOutput=True)

    with ExitStack() as ctx:
        tc = ctx.enter_context(tile.TileContext(nc))
        kpool = ctx.enter_context(tc.tile_pool(name="kmat", bufs=1))
        vpool = ctx.enter_context(tc.tile_pool(name="vec", bufs=1))

        # --- params / constants -----------------------------------------
        wt_sb = vpool.tile([128, 64], F32, tag="wt", name="wt")
        nc.sync.dma_start(out=wt_sb[:], in_=wtsp[:])
        aw_v = wt_sb[:, 0:16].rearrange("p (c e) -> p c e", c=4)
        bw_v = wt_sb[:, 16:32].rearrange("p (c e) -> p c e", c=4)
        ones = vpool.tile([128, 1], F32, tag="ones", name="ones")
        nc.gpsimd.memset(ones[:], 1.0)
        sel = vpool.tile([128, 4], BF16, tag="sel", name="sel")
        nc.gpsimd.memset(sel[:], 0.0)
        for e in range(ELEMS):
            nc.gpsimd.memset(sel[32 * e:32 * e + 1, e:e + 1], 1.0)
        outsb = vpool.tile([1, ELEMS], F32, tag="outsb", name="outsb")
        bias12 = vpool.tile([128, 1], F32, tag="bias12", name="bias12")
        nc.gpsimd.memset(bias12[:], 1e-12)

        # potentials: [128, (c, e, 32)] bf16; col 0 of each 32-block is the
        # live value, cols 1-31 stay zero (zero-pads the M=32 stationary,
        # which also zero-fills the unused PSUM rows).
        U_all = vpool.tile([128, 4, 4, 32], BF16, tag="U", name="U")
        V_all = vpool.tile([128, 4, 4, 32], BF16, tag="V", name="V")
        nc.gpsimd.memset(U_all[:], 0.0)
        nc.gpsimd.memset(V_all[:], 0.0)

        KA, KB, DK, AB_SB = {}, {}, {}, {}
        for e in range(ELEMS):
            ab_sb = vpool.tile([4, 2 * N], mybir.dt.float32r,
                               tag=f"ABs{e}", name=f"ABs{e}")
            nc.sync.dma_start(out=ab_sb[:], in_=ABaug[e])
            AB_SB[e] = (ab_sb[:, 0:N], ab_sb[:, N:2 * N])
            KA[e], KB[e], DK[e] = [], [], []
            for c in range(4):
                KA[e].append(kpool.tile([128, 512], BF16, tag=f"KA{e}{c}",
                                        name=f"KA{e}{c}"))
                KB[e].append(kpool.tile([128, 512], BF16, tag=f"KB{e}{c}",
                                        name=f"KB{e}{c}"))
                DK[e].append(kpool.tile([128, 512], BF16, tag=f"DK{e}{c}",
                                        name=f"DK{e}{c}"))

        # --- setup: K'A = aw*K, K'B = bw*K, DK' = d2*aw*K ----------------
        with tc.tile_pool(name="pd2", bufs=2, space="PSUM") as pd2, \
             tc.tile_pool(name="cl", bufs=1) as clpool, \
             tc.tile_pool(name="st", bufs=1) as stpool:
            CLA, ST = {}, {}
            last_sqrt = None
            for e in range(ELEMS):
                a_sb, b_sb = AB_SB[e]
                for side in ("A", "B"):
                    Lt, Rt = (a_sb, b_sb) if side == "A" else (b_sb, a_sb)
                    for c in range(4):
                        d2 = pd2.tile([128, 512], F32, tag="d2", name="d2")
                        nc.tensor.matmul(
                            d2[:], Lt[:, c * 128:(c + 1) * 128], Rt[:],
                            start=True, stop=True,
                        )
                        st = stpool.tile([128, 512], F32,
                                         tag=f"st{e}{side}{c}",
                                         name=f"st{e}{side}{c}")
                        if side == "A":
                            cl = clpool.tile([128, 512], F32, tag=f"cl{e}{c}",
                                             name=f"cl{e}{c}")
                            CLA[(e, c)] = cl
                        else:
                            cl = clpool.tile([128, 512], F32, tag="clB",
                                             name="clB", bufs=2)
                        nc.vector.tensor_scalar_max(cl[:], d2[:], 0.0)
                        last_sqrt = nc.scalar.activation(
                            st[:], cl[:], AF.Sqrt, bias=bias12[:]
                        )
                        ST[(e, side, c)] = st
            # all exps after all sqrts (ACT table sets differ; interleaving
            # reloads ~1.3us per switch)
            for e in range(ELEMS):
                for side in ("A", "B"):
                    for c in range(4):
                        bias_col = (32 if side == "A" else 48) + 4 * c + e
                        kt = (KA if side == "A" else KB)[e][c]
                        exp_inst = nc.scalar.activation(
                            kt[:], ST[(e, side, c)][:], AF.Exp,
                            scale=-1.0 / EPS,
                            bias=wt_sb[:, bias_col:bias_col + 1],
                        )
                        tile.add_dep_helper(exp_inst.ins, last_sqrt.ins,
                                            sync=True,
                                            reason="act-table-batch")
            # DK' = clamp(d2) * K'A   (st^2 == cl + 1e-12; 1e-12 is noise)
            for e in range(ELEMS):
                for c in range(4):
                    nc.vector.tensor_mul(DK[e][c][:], CLA[(e, c)][:],
                                         KA[e][c][:])
            # first u-update: U~1 = aw / rowsum(K'A)  (v0 = 1 incl. masked)
            rs = vpool.tile([128, 16], F32, tag="rs", name="rs")
            for e in range(ELEMS):
                for c in range(4):
                    nc.vector.reduce_sum(rs[:, 4 * c + e:4 * c + e + 1],
                                         KA[e][c][:],
                                         axis=mybir.AxisListType.X)
            rs2 = vpool.tile([128, 16], F32, tag="rs2", name="rs2")
            nc.vector.tensor_scalar_max(rs2[:], rs[:], 1e-30)
            rcp = vpool.tile([128, 16], F32, tag="rcp", name="rcp")
            nc.vector.reciprocal(rcp[:], rs2[:])
            nc.vector.tensor_mul(
                U_all[:, :, :, 0],
                rcp[:].rearrange("p (c e) -> p c e", c=4), aw_v,
            )

        # --- Sinkhorn iterations ----------------------------------------
        with tc.tile_pool(name="ps", bufs=2, space="PSUM") as pspool, \
             tc.tile_pool(name="pt16", bufs=2, space="PSUM") as ptpool, \
             tc.tile_pool(name="dps", bufs=1, space="PSUM") as dpool, \
             tc.tile_pool(name="sb", bufs=2) as sbpool:

            dummy_ps = dpool.tile([1, 512], F32, tag="dps", name="dps")

            def dummy(n):
                for _ in range(n):
                    nc.tensor.matmul(
                        dummy_ps[:, 0:64], sel[:, 0:1], KA[0][0][:, 0:64],
                        start=True, stop=True, skip_group_check=True,
                    )

            # warm-up burst: ~4us of back-to-back matmuls flips the HAM
            # clock gate to 8/8 before the loop
            for _ in range(9):
                nc.tensor.matmul(
                    dummy_ps[:], sel[:, 0:1], KA[0][0][:],
                    start=True, stop=True, skip_group_check=True,
                )

            def phase(Kt, stat, out_all):
                """out_all[:,c,e,0] = 1 / (sum_c stat[:,c,e,:]^T @ Kt)"""
                ps = pspool.tile([128, 512], F32, tag="ps", name="ps")
                for c in range(4):
                    for e in range(ELEMS):
                        nc.tensor.matmul(
                            ps[32 * e:32 * e + 32, :],
                            stat[:, c, e, :], Kt[e][c][:],
                            start=(c == 0), stop=(c == 3),
                            tile_position=(0, 32 * e),
                        )
                dummy(6)
                sbu = sbpool.tile([128, 512], BF16, tag="sbu", name="sbu")
                nc.vector.tensor_copy(sbu[:, 0:256], ps[:, 0:256])
                nc.scalar.copy(sbu[:, 256:512], ps[:, 256:512])
                pt16 = ptpool.tile([128, 4, 4], F32, tag="pt16", name="pt16")
                for c in range(4):
                    nc.tensor.matmul(
                        pt16[:, c, :], sbu[:, c * 128:(c + 1) * 128], sel[:],
                        start=True, stop=True,
                    )
                dummy(4)
                with nc.allow_low_precision("bf16 Sinkhorn potentials"):
                    nc.vector.reciprocal(out_all[:, :, :, 0], pt16[:])
                return pt16

            # reference order: 50x(u-update; v-update). u#1 done at setup.
            for _ in range(ITERS - 1):
                phase(KA, U_all, V_all)   # v-update
                phase(KB, V_all, U_all)   # u-update
            phase(KA, U_all, V_all)       # final v-update

            # --- final: ot[e] = (DK'^T U~) . (bw * V~) ------------------
            ps = pspool.tile([128, 512], F32, tag="ps", name="ps")
            for c in range(4):
                for e in range(ELEMS):
                    nc.tensor.matmul(
                        ps[32 * e:32 * e + 32, :],
                        U_all[:, c, e, :], DK[e][c][:],
                        start=(c == 0), stop=(c == 3),
                        tile_position=(0, 32 * e),
                    )
            sbg = sbpool.tile([128, 512], BF16, tag="sbu", name="sbg")
            nc.vector.tensor_copy(sbg[:, 0:256], ps[:, 0:256])
            nc.scalar.copy(sbg[:, 256:512], ps[:, 256:512])
            ptg = ptpool.tile([128, 4, 4], F32, tag="pt16", name="ptg")
            for c in range(4):
                nc.tensor.matmul(
                    ptg[:, c, :], sbg[:, c * 128:(c + 1) * 128], sel[:],
                    start=True, stop=True,
                )
            t1 = sbpool.tile([128, 4, 4], F32, tag="t1", name="t1")
            nc.vector.tensor_mul(t1[:], ptg[:], V_all[:, :, :, 0])
            t2 = sbpool.tile([128, 4, 4], F32, tag="t2", name="t2")
            nc.vector.tensor_mul(t2[:], t1[:], bw_v)
            r_e = sbpool.tile([128, 4], F32, tag="re", name="re")
            nc.vector.reduce_sum(
                r_e[:], t2[:].rearrange("p c e -> p e c"),
                axis=mybir.AxisListType.X,
            )
            po = ptpool.tile([1, 4], F32, tag="pt16", name="po")
            nc.tensor.matmul(po[:], ones[:], r_e[:], start=True, stop=True)
            nc.scalar.copy(outsb[:], po[:])
            nc.sync.dma_start(out=otp[:], in_=outsb[:])
    nc.compile()
    return nc


_NC_CACHE = {}


def _get_nc():
    if "nc" not in _NC_CACHE:
        _NC_CACHE["nc"] = _build_nc()
    return _NC_CACHE["nc"]


def _host_prep(a_mask, pc_a, b_mask, pc_b):
    """Per-batch-element f32 prep mirroring the reference's masking."""
    f32 = np.float32
    a_pt = (a_mask * pc_a[..., 2]).astype(f32)          # [B,N]
    b_pt = (b_mask * pc_b[..., 2]).astype(f32)          # [B,M]
    va = (a_pt > 0).astype(f32)
    vb = (b_pt > 0).astype(f32)
    aw = (a_pt / a_pt.sum(axis=1, keepdims=True, dtype=f32)).astype(f32)
    bw = (b_pt / b_pt.sum(axis=1, keepdims=True, dtype=f32)).astype(f32)
    xa = pc_a[..., :2].astype(f32)                      # [B,N,2]
    xb = pc_b[..., :2].astype(f32)
    onesN = np.ones((B, N), f32)
    A = np.stack(
        [-2 * xa[..., 0], -2 * xa[..., 1],
         (xa * xa).sum(-1).astype(f32), onesN], axis=1
    ) * va[:, None, :]                                  # [B,4,N]
    Bm = np.stack(
        [xb[..., 0], xb[..., 1], onesN,
         (xb * xb).sum(-1).astype(f32)], axis=1
    ) * vb[:, None, :]                                  # [B,4,M]
    # huber term on host (tiny)
    e = (a_pt.sum(axis=1, dtype=f32) - b_pt.sum(axis=1, dtype=f32)).astype(f32)
    hub = np.where(np.abs(e) <= 1.0, f32(0.5) * e * e, np.abs(e) - f32(0.5))
    with np.errstate(divide="ignore"):
        ln_aw = np.where(aw > 0, np.log(np.where(aw > 0, aw, 1.0)),
                         f32(-1e9)).astype(f32)
        ln_bw = np.where(bw > 0, np.log(np.where(bw > 0, bw, 1.0)),
                         f32(-1e9)).astype(f32)
    # [B, 4, 128]: chunk-major weights
    chunk = lambda x: x.reshape(B, 4, 128).astype(f32)
    AB = np.concatenate([A.astype(f32), Bm.astype(f32)], axis=2)  # [B,4,1024]
    return (AB, chunk(aw), chunk(bw), chunk(ln_aw), chunk(ln_bw),
            hub.astype(f32))


def kernel(a_mask, pc_a, b_mask, pc_b, _trace=False):
    AB, aw_pm, bw_pm, lna_pm, lnb_pm, hub = _host_prep(
        np.asarray(a_mask), np.asarray(pc_a), np.asarray(b_mask), np.asarray(pc_b)
    )
    in_maps = []
    for core in range(N_CORES):
        sl = slice(core * ELEMS, (core + 1) * ELEMS)
        # [p, (c, e)] layout per weight
        cols = [x[sl].transpose(2, 1, 0).reshape(128, 16)
                for x in (aw_pm, bw_pm, lna_pm, lnb_pm)]
        in_maps.append({
            "ABaug": np.ascontiguousarray(AB[sl]),
            "wts": np.ascontiguousarray(np.concatenate(cols, axis=1)),
        })
    nc = _get_nc()
    res = run_bass_kernel_spmd(nc, in_maps, list(range(N_CORES)), trace=_trace)
    ot = np.concatenate([res.results[c]["ot"].reshape(ELEMS) for c in range(N_CORES)])
    out = (ot + hub).astype(np.float32)
    if _trace:
        return out, res
    return out


# revision 16
# speedup vs baseline: 1.5064x; 1.0226x over previous
"""Trainium2 Bass kernel for nn_EnergyMovers (batched Sinkhorn OT loss).

Strategy (pure data parallelism, 4 batch elems per core x 8 cores):
  - Host: build masked augmented point vectors so d2[n,m] = sum_k A[k,n]*B[k,m]
    comes out of a K=4 TensorE matmul already masked (masked rows/cols -> d2=0
    -> K=exp(-sqrt(1e-12)/eps) ~ 1, matching the reference's logK=0 there).
  - Device per elem: d2 (both layouts) -> clamp(DVE) -> sqrt(ACT) -> exp(ACT)
    with the marginal weights FOLDED INTO THE KERNEL MATRICES via the exp
    bias: K'A = exp(-d/eps + ln(aw_n)) = aw_n * K (layout [n,m]),
    K'B = bw_m * K (layout [m,n]), DK' = clamp(d2) * K'A = aw_n * d^2 * K.
  - Non-log Sinkhorn on reciprocal potentials (u = aw*U~, v = bw*V~):
        s_v = K'A^T @ U~ ; V~ = 1/s_v ; s_u = K'B^T @ V~ ; U~ = 1/s_u
    identical to the reference's log-domain iteration (f32/bf16 exponent
    range suffices). The first u-update is the row-sum of K'A (v0 = 1
    including the reference's masked-column exp(0)=1 terms), done at setup
    with DVE reductions.
  - Each matvec is COLUMN-TILED on the PE: the 4 batch elems go to the 4
    32-wide PE column groups (tile_position (0,32e)), so their K-matrix
    streams run concurrently. The M=32 stationary has the potential chunk in
    col 0, zeros elsewhere (also zero-fills unused PSUM rows).
  - Row -> partition-major conversion via SELECTOR MATMULS: pt[128,4] =
    sbu_chunk^T @ sel where sel[32e,e]=1 picks the 4 result rows. Then ONE
    strided DVE reciprocal [128,(c,e)] produces the next stationaries.
  - Dummy N=64 matmuls pad the PE pipeline during DVE/ACT tail waits so the
    HAM clock gate sees sustained activity and keeps the PE at 2.4 GHz.
  - Final: ot[e] = (DK'^T U~) . (bw * V~) via one more column-tiled matvec +
    selector matmuls + DVE muls + ones-matmul partition reduction.
  - Host: huber(e) added, results gathered from 8 cores.
"""

import os
from contextlib import ExitStack

import numpy as np

import concourse.bass as bass
import concourse.bacc as bacc
import concourse.mybir as mybir
import concourse.tile as tile
from concourse.bass_utils import run_bass_kernel_spmd

N_CORES = 8
ELEMS = 4  # batch elements per core (B=32 / 8)
B, N, M = 32, 512, 512
EPS = 0.05
ITERS = int(os.environ.get("EM_ITERS", "50"))
F32 = mybir.dt.float32
BF16 = mybir.dt.bfloat16
AF = mybir.ActivationFunctionType
ALU = mybir.AluOpType


def _build_nc():
    nc = bacc.Bacc()
    ABaug = nc.declare_dram_parameter("ABaug", [ELEMS, 4, 2 * N],
                                      mybir.dt.float32r, isOutput=False)
    # wts cols: 0:16 aw[(c,e)], 16:32 bw[(c,e)], 32:48 ln_aw, 48:64 ln_bw
    wtsp = nc.declare_dram_parameter("wts", [128, 64], F32, isOutput=False)
    otp = nc.declare_dram_parameter("ot", [1, ELEMS], F32, isOutput=True)
    # keeps the HAM warm-up / filler matmuls live through dead-code elim
    scrp = nc.declare_dram_parameter("scr", [1, 1], F32, isOutput=True)

    with ExitStack() as ctx:
        tc = ctx.enter_context(tile.TileContext(nc))
        kpool = ctx.enter_context(tc.tile_pool(name="kmat", bufs=1))
        vpool = ctx.enter_context(tc.tile_pool(name="vec", bufs=1))

        # --- params / constants -----------------------------------------
        wt_sb = vpool.tile([128, 64], F32, tag="wt", name="wt")
        nc.sync.dma_start(out=wt_sb[:], in_=wtsp[:])
        aw_v = wt_sb[:, 0:16].rearrange("p (c e) -> p c e", c=4)
        bw_v = wt_sb[:, 16:32].rearrange("p (c e) -> p c e", c=4)
        ones = vpool.tile([128, 1], F32, tag="ones", name="ones")
        nc.gpsimd.memset(ones[:], 1.0)
        sel = vpool.tile([128, 4], BF16, tag="sel", name="sel")
        nc.gpsimd.memset(sel[:], 0.0)
        for e in range(ELEMS):
            nc.gpsimd.memset(sel[32 * e:32 * e + 1, e:e + 1], 1.0)
        outsb = vpool.tile([1, ELEMS], F32, tag="outsb", name="outsb")
        bias12 = vpool.tile([128, 1], F32, tag="bias12", name="bias12")
        nc.gpsimd.memset(bias12[:], 1e-12)

        # potentials: [128, (c, e, 32)] bf16; col 0 of each 32-block is the
        # live value, cols 1-31 stay zero (zero-pads the M=32 stationary,
        # which also zero-fills the unused PSUM rows).
        U_all = vpool.tile([128, 4, 4, 32], BF16, tag="U", name="U")
        V_all = vpool.tile([128, 4, 4, 32], BF16, tag="V", name="V")
        nc.gpsimd.memset(U_all[:], 0.0)
        nc.gpsimd.memset(V_all[:], 0.0)

        KA, KB, DK, AB_SB = {}, {}, {}, {}
        for e in range(ELEMS):
            ab_sb = vpool.tile([4, 2 * N], mybir.dt.float32r,
                               tag=f"ABs{e}", name=f"ABs{e}")
            nc.sync.dma_start(out=ab_sb[:], in_=ABaug[e])
            AB_SB[e] = (ab_sb[:, 0:N], ab_sb[:, N:2 * N])
            KA[e], KB[e], DK[e] = [], [], []
            for c in range(4):
                KA[e].append(kpool.tile([128, 512], BF16, tag=f"KA{e}{c}",
                                        name=f"KA{e}{c}"))
                KB[e].append(kpool.tile([128, 512], BF16, tag=f"KB{e}{c}",
                                        name=f"KB{e}{c}"))
                DK[e].append(kpool.tile([128, 512], BF16, tag=f"DK{e}{c}",
                                        name=f"DK{e}{c}"))

        # --- setup: K'A = aw*K, K'B = bw*K, DK' = d2*aw*K ----------------
        with tc.tile_pool(name="pd2", bufs=2, space="PSUM") as pd2, \
             tc.tile_pool(name="cl", bufs=1) as clpool, \
             tc.tile_pool(name="st", bufs=1) as stpool:
            CLA, ST = {}, {}
            last_sqrt = None
            for e in range(ELEMS):
                a_sb, b_sb = AB_SB[e]
                for side in ("A", "B"):
                    Lt, Rt = (a_sb, b_sb) if side == "A" else (b_sb, a_sb)
                    for c in range(4):
                        d2 = pd2.tile([128, 512], F32, tag="d2", name="d2")
                        nc.tensor.matmul(
                            d2[:], Lt[:, c * 128:(c + 1) * 128], Rt[:],
                            start=True, stop=True,
                        )
                        st = stpool.tile([128, 512], F32,
                                         tag=f"st{e}{side}{c}",
                                         name=f"st{e}{side}{c}")
                        if side == "A":
                            cl = clpool.tile([128, 512], F32, tag=f"cl{e}{c}",
                                             name=f"cl{e}{c}")
                            CLA[(e, c)] = cl
                        else:
                            cl = clpool.tile([128, 512], F32, tag="clB",
                                             name="clB", bufs=2)
                        nc.vector.tensor_scalar_max(cl[:], d2[:], 0.0)
                        last_sqrt = nc.scalar.activation(
                            st[:], cl[:], AF.Sqrt, bias=bias12[:]
                        )
                        ST[(e, side, c)] = st
            # all exps after all sqrts (ACT table sets differ; interleaving
            # reloads ~1.3us per switch)
            for e in range(ELEMS):
                for side in ("A", "B"):
                    for c in range(4):
                        bias_col = (32 if side == "A" else 48) + 4 * c + e
                        kt = (KA if side == "A" else KB)[e][c]
                        exp_inst = nc.scalar.activation(
                            kt[:], ST[(e, side, c)][:], AF.Exp,
                            scale=-1.0 / EPS,
                            bias=wt_sb[:, bias_col:bias_col + 1],
                        )
                        tile.add_dep_helper(exp_inst.ins, last_sqrt.ins,
                                            sync=True,
                                            reason="act-table-batch")
            # DK' = clamp(d2) * K'A   (st^2 == cl + 1e-12; 1e-12 is noise)
            for e in range(ELEMS):
                for c in range(4):
                    nc.vector.tensor_mul(DK[e][c][:], CLA[(e, c)][:],
                                         KA[e][c][:])
            # first u-update: U~1 = aw / rowsum(K'A)  (v0 = 1 incl. masked)
            rs = vpool.tile([128, 16], F32, tag="rs", name="rs")
            for e in range(ELEMS):
                for c in range(4):
                    nc.vector.reduce_sum(rs[:, 4 * c + e:4 * c + e + 1],
                                         KA[e][c][:],
                                         axis=mybir.AxisListType.X)
            rs2 = vpool.tile([128, 16], F32, tag="rs2", name="rs2")
            nc.vector.tensor_scalar_max(rs2[:], rs[:], 1e-30)
            rcp = vpool.tile([128, 16], F32, tag="rcp", name="rcp")
            nc.vector.reciprocal(rcp[:], rs2[:])
            nc.vector.tensor_mul(
                U_all[:, :, :, 0],
                rcp[:].rearrange("p (c e) -> p c e", c=4), aw_v,
            )

        # --- Sinkhorn iterations ----------------------------------------
        with tc.tile_pool(name="ps", bufs=2, space="PSUM") as pspool, \
             tc.tile_pool(name="pt16", bufs=2, space="PSUM") as ptpool, \
             tc.tile_pool(name="dps", bufs=1, space="PSUM") as dpool, \
             tc.tile_pool(name="sb", bufs=2) as sbpool:

            dummy_ps = dpool.tile([1, 512], F32, tag="dps", name="dps")

            def dummy(n):
                for _ in range(n):
                    nc.tensor.matmul(
                        dummy_ps[:, 0:64], sel[:, 0:1], KA[0][0][:, 0:64],
                        start=True, stop=True, skip_group_check=True,
                    )

            # warm-up burst: ~4us of back-to-back matmuls flips the HAM
            # clock gate to 8/8 before the loop
            for _ in range(9):
                nc.tensor.matmul(
                    dummy_ps[:], sel[:, 0:1], KA[0][0][:],
                    start=True, stop=True, skip_group_check=True,
                )

            def phase(Kt, stat, out_all):
                """out_all[:,c,e,0] = 1 / (sum_c stat[:,c,e,:]^T @ Kt)"""
                ps = pspool.tile([128, 512], F32, tag="ps", name="ps")
                for c in range(4):
                    for e in range(ELEMS):
                        nc.tensor.matmul(
                            ps[32 * e:32 * e + 32, :],
                            stat[:, c, e, :], Kt[e][c][:],
                            start=(c == 0), stop=(c == 3),
                            tile_position=(0, 32 * e),
                        )
                dummy(6)
                sbu0 = sbpool.tile([128, 256], BF16, tag="sbu0", name="sbu0")
                sbu1 = sbpool.tile([128, 256], BF16, tag="sbu1", name="sbu1")
                nc.vector.tensor_copy(sbu0[:], ps[:, 0:256])
                nc.scalar.copy(sbu1[:], ps[:, 256:512])
                pt16 = ptpool.tile([128, 4, 4], F32, tag="pt16", name="pt16")
                for c in range(4):
                    src = sbu0 if c < 2 else sbu1
                    nc.tensor.matmul(
                        pt16[:, c, :], src[:, (c % 2) * 128:(c % 2 + 1) * 128],
                        sel[:], start=True, stop=True,
                    )
                dummy(4)
                with nc.allow_low_precision("bf16 Sinkhorn potentials"):
                    nc.vector.reciprocal(out_all[:, :, :, 0], pt16[:])
                return pt16

            # reference order: 50x(u-update; v-update). u#1 done at setup.
            for _ in range(ITERS - 1):
                phase(KA, U_all, V_all)   # v-update
                phase(KB, V_all, U_all)   # u-update
            phase(KA, U_all, V_all)       # final v-update

            # --- final: ot[e] = (DK'^T U~) . (bw * V~) ------------------
            ps = pspool.tile([128, 512], F32, tag="ps", name="ps")
            for c in range(4):
                for e in range(ELEMS):
                    nc.tensor.matmul(
                        ps[32 * e:32 * e + 32, :],
                        U_all[:, c, e, :], DK[e][c][:],
                        start=(c == 0), stop=(c == 3),
                        tile_position=(0, 32 * e),
                    )
            sbg0 = sbpool.tile([128, 256], BF16, tag="sbu0", name="sbg0")
            sbg1 = sbpool.tile([128, 256], BF16, tag="sbu1", name="sbg1")
            nc.vector.tensor_copy(sbg0[:], ps[:, 0:256])
            nc.scalar.copy(sbg1[:], ps[:, 256:512])
            ptg = ptpool.tile([128, 4, 4], F32, tag="pt16", name="ptg")
            for c in range(4):
                src = sbg0 if c < 2 else sbg1
                nc.tensor.matmul(
                    ptg[:, c, :], src[:, (c % 2) * 128:(c % 2 + 1) * 128],
                    sel[:], start=True, stop=True,
                )
            t1 = sbpool.tile([128, 4, 4], F32, tag="t1", name="t1")
            nc.vector.tensor_mul(t1[:], ptg[:], V_all[:, :, :, 0])
            t2 = sbpool.tile([128, 4, 4], F32, tag="t2", name="t2")
            nc.vector.tensor_mul(t2[:], t1[:], bw_v)
            r_e = sbpool.tile([128, 4], F32, tag="re", name="re")
            nc.vector.reduce_sum(
                r_e[:], t2[:].rearrange("p c e -> p e c"),
                axis=mybir.AxisListType.X,
            )
            po = ptpool.tile([1, 4], F32, tag="pt16", name="po")
            nc.tensor.matmul(po[:], ones[:], r_e[:], start=True, stop=True)
            nc.scalar.copy(outsb[:], po[:])
            nc.sync.dma_start(out=otp[:], in_=outsb[:])
            # keep the dummy matmuls live (HAM warmth depends on them)
            scr_sb = vpool.tile([1, 1], F32, tag="scr", name="scr")
            nc.scalar.copy(scr_sb[:], dummy_ps[0:1, 0:1])
            nc.sync.dma_start(out=scrp[:], in_=scr_sb[:])
    nc.compile()
    return nc


_NC_CACHE = {}


def _get_nc():
    if "nc" not in _NC_CACHE:
        _NC_CACHE["nc"] = _build_nc()
    return _NC_CACHE["nc"]


def _host_prep(a_mask, pc_a, b_mask, pc_b):
    """Per-batch-element f32 prep mirroring the reference's masking."""
    f32 = np.float32
    a_pt = (a_mask * pc_a[..., 2]).astype(f32)          # [B,N]
    b_pt = (b_mask * pc_b[..., 2]).astype(f32)          # [B,M]
    va = (a_pt > 0).astype(f32)
    vb = (b_pt > 0).astype(f32)
    aw = (a_pt / a_pt.sum(axis=1, keepdims=True, dtype=f32)).astype(f32)
    bw = (b_pt / b_pt.sum(axis=1, keepdims=True, dtype=f32)).astype(f32)
    xa = pc_a[..., :2].astype(f32)                      # [B,N,2]
    xb = pc_b[..., :2].astype(f32)
    onesN = np.ones((B, N), f32)
    A = np.stack(
        [-2 * xa[..., 0], -2 * xa[..., 1],
         (xa * xa).sum(-1).astype(f32), onesN], axis=1
    ) * va[:, None, :]                                  # [B,4,N]
    Bm = np.stack(
        [xb[..., 0], xb[..., 1], onesN,
         (xb * xb).sum(-1).astype(f32)], axis=1
    ) * vb[:, None, :]                                  # [B,4,M]
    # huber term on host (tiny)
    e = (a_pt.sum(axis=1, dtype=f32) - b_pt.sum(axis=1, dtype=f32)).astype(f32)
    hub = np.where(np.abs(e) <= 1.0, f32(0.5) * e * e, np.abs(e) - f32(0.5))
    with np.errstate(divide="ignore"):
        ln_aw = np.where(aw > 0, np.log(np.where(aw > 0, aw, 1.0)),
                         f32(-1e9)).astype(f32)
        ln_bw = np.where(bw > 0, np.log(np.where(bw > 0, bw, 1.0)),
                         f32(-1e9)).astype(f32)
    # [B, 4, 128]: chunk-major weights
    chunk = lambda x: x.reshape(B, 4, 128).astype(f32)
    AB = np.concatenate([A.astype(f32), Bm.astype(f32)], axis=2)  # [B,4,1024]
    return (AB, chunk(aw), chunk(bw), chunk(ln_aw), chunk(ln_bw),
            hub.astype(f32))


def kernel(a_mask, pc_a, b_mask, pc_b, _trace=False):
    AB, aw_pm, bw_pm, lna_pm, lnb_pm, hub = _host_prep(
        np.asarray(a_mask), np.asarray(pc_a), np.asarray(b_mask), np.asarray(pc_b)
    )
    in_maps = []
    for core in range(N_CORES):
        sl = slice(core * ELEMS, (core + 1) * ELEMS)
        # [p, (c, e)] layout per weight
        cols = [x[sl].transpose(2, 1, 0).reshape(128, 16)
                for x in (aw_pm, bw_pm, lna_pm, lnb_pm)]
        in_maps.append({
            "ABaug": np.ascontiguousarray(AB[sl]),
            "wts": np.ascontiguousarray(np.concatenate(cols, axis=1)),
        })
    nc = _get_nc()
    res = run_bass_kernel_spmd(nc, in_maps, list(range(N_CORES)), trace=_trace)
    ot = np.concatenate([res.results[c]["ot"].reshape(ELEMS) for c in range(N_CORES)])
    out = (ot + hub).astype(np.float32)
    if _trace:
        return out, res
    return out
